# revision 12
# baseline (speedup 1.0000x reference)
"""Multi-head causal attention (B=4, T=2048, E=1024, H=16) on 8 TRN2 NeuronCores.

Sharding: core c handles batch b = c//2 and head-group g = c%2 (8 heads = 512
of the 1024 embedding dims). Each core runs an independent single-core kernel:

  QT = (Wq_g @ xq.T)        [512, T]   (d on partitions, 4 strips of 128)
  KT = (Wk_g @ xkv.T)       [512, T]
  V  = (xkv @ Wv_g.T)       [T, 512]   (t on partitions, + ones column -> VE)
  per (tq-chunk c512, head h):
     S.T[tk_blk j, tq] = KT_h[:, j].T @ QT_h[:, c512]   (K=64 matmul)
     P.T = exp(S.T / 8) * causal_mask                    (ScalarE + DVE)
     O.T[65, 512] += [V_h | 1][tk_blk].T @ P.T           (PSUM accumulate)
     O = transpose(O.T); out = O[:, :64] / O[:, 64]      (PE + DVE)

Inputs are pre-transposed and bf16-cast on the host; matmuls are bf16 with
fp32 PSUM accumulation; softmax runs unnormalized exp (scores are O(1) by
construction) with the denominator from the appended ones column.
"""

import os
import numpy as np
import ml_dtypes

import concourse.bass as bass
import concourse.bacc as bacc
import concourse.mybir as mybir
import concourse.tile as tile
from concourse.bass_utils import run_bass_kernel_spmd
from concourse.masks import make_identity

F32 = mybir.dt.float32
BF16 = mybir.dt.bfloat16

P = 128  # partitions
D = 64  # head dim
B, T_FULL, E, H_TOT = 4, 2048, 1024, 16
HLOC = 8  # heads per core
DLOC = HLOC * D  # 512: local slice of E
N_CORES = 8


def build(T=T_FULL):
    """Single-core graph; same graph runs SPMD on all 8 cores."""
    assert T % 512 == 0
    TC = T // 512  # tq chunks of 512
    NTB = T // P  # tk blocks of 128
    KCH = E // P  # 8 contraction chunks for projections
    MCH = DLOC // P  # 4 output strips for QT/KT

    nc = bacc.Bacc("TRN2", target_bir_lowering=False, debug=False,
                   num_devices=N_CORES)

    xqT = nc.dram_tensor("xqT", [E, T], BF16, kind="ExternalInput")
    xkvT = nc.dram_tensor("xkvT", [E, T], BF16, kind="ExternalInput")
    wqT = nc.dram_tensor("wqT", [E, DLOC], BF16, kind="ExternalInput")
    wkT = nc.dram_tensor("wkT", [E, DLOC], BF16, kind="ExternalInput")
    wvT = nc.dram_tensor("wvT", [E, DLOC], BF16, kind="ExternalInput")
    out = nc.dram_tensor("out", [T, DLOC], F32, kind="ExternalOutput")

    xqT_v = xqT.ap().rearrange("(k p) t -> p k t", p=P)
    xkvT_v = xkvT.ap().rearrange("(k p) t -> p k t", p=P)

    with tile.TileContext(nc) as tc:
        with (
            tc.tile_pool(name="persist", bufs=1) as persist,
            tc.tile_pool(name="wpool", bufs=3) as wpool,
            tc.tile_pool(name="xpool", bufs=3) as xpool,
            tc.tile_pool(name="lpool", bufs=5) as lpool,
            tc.tile_pool(name="ptpool", bufs=20) as ptpool,
            tc.tile_pool(name="otpool", bufs=3) as otpool,
            tc.tile_pool(name="osb", bufs=2) as osb,
            tc.tile_pool(name="rpool", bufs=8) as rpool,
            tc.tile_pool(name="mm_ps", bufs=4, space="PSUM") as mm_ps,
            tc.tile_pool(name="pv_ps", bufs=2, space="PSUM") as pv_ps,
            tc.tile_pool(name="ot_ps", bufs=2, space="PSUM") as ot_ps,
        ):
            # ---- constants ----
            ident = persist.tile([P, P], F32, tag="ident")
            make_identity(nc, ident[:])
            # masks4[:, r, :]: cols [0,128r) = 0, cols [128r,128r+128) =
            # upper triangle (keep col >= row), rest = 1
            masks4 = persist.tile([P, 4, 512], BF16, tag="masks4")
            nc.gpsimd.memset(masks4[:], 1.0)
            for r in range(4):
                if r > 0:
                    nc.gpsimd.memset(masks4[:, r, 0 : P * r], 0.0)
                nc.gpsimd.affine_select(
                    out=masks4[:, r, P * r : P * r + P],
                    in_=masks4[:, r, P * r : P * r + P],
                    compare_op=mybir.AluOpType.is_ge,
                    fill=0.0,
                    base=0,
                    pattern=[[1, P]],
                    channel_multiplier=-1,
                )

            QT = persist.tile([P, MCH, T], BF16, tag="QT")
            KT = persist.tile([P, MCH, T], BF16, tag="KT")
            VE = persist.tile([P, NTB, HLOC, D + 1], BF16, tag="VE")

            # weights resident for all projection slices
            wts = {}
            for nm, wdram in (("q", wqT), ("k", wkT), ("v", wvT)):
                wt = wpool.tile([P, KCH, DLOC], BF16, tag="w", name=f"w{nm}")
                for k in range(KCH):
                    nc.sync.dma_start(
                        wt[:, k, :], wdram.ap()[P * k : P * k + P, :]
                    )
                wts[nm] = wt

            def emit_proj_slice(n):
                """QT/KT strips and VE blocks for t in [512n, 512(n+1))."""
                for nm, dst, xv in (
                    ("q", QT, xqT_v),
                    ("k", KT, xkvT_v),
                ):
                    wt = wts[nm]
                    xt = xpool.tile(
                        [P, KCH, 512], BF16, tag="x", name=f"x{nm}{n}"
                    )
                    for k in range(KCH):
                        nc.sync.dma_start(
                            xt[:, k, :], xv[:, k, 512 * n : 512 * n + 512]
                        )
                    for m in range(MCH):
                        ps = mm_ps.tile([P, 512], F32, tag="s")
                        for k in range(KCH):
                            nc.tensor.matmul(
                                ps[:],
                                wt[:, k, P * m : P * m + P],
                                xt[:, k, :],
                                start=(k == 0),
                                stop=(k == KCH - 1),
                            )
                        nc.vector.tensor_copy(
                            dst[:, m, 512 * n : 512 * n + 512], ps[:]
                        )
                wv = wts["v"]
                for i in range(4 * n, 4 * n + 4):
                    nc.vector.memset(VE[:, i, :, D : D + 1], 1.0)
                    lt = lpool.tile([P, KCH, P], BF16, tag="l", name=f"l{i}")
                    for k in range(KCH):
                        nc.sync.dma_start(
                            lt[:, k, :], xkvT_v[:, k, P * i : P * i + P]
                        )
                    ps = mm_ps.tile([P, 512], F32, tag="s")
                    for k in range(KCH):
                        nc.tensor.matmul(
                            ps[:],
                            lt[:, k, :],
                            wv[:, k, :],
                            start=(k == 0),
                            stop=(k == KCH - 1),
                        )
                    nc.vector.tensor_copy(
                        VE[:, i, :, 0:D],
                        ps[:].rearrange("p (h d) -> p h d", h=HLOC),
                    )

            # ---- attention, software-pipelined over (c, h), with the
            # next projection slice interleaved mid-chunk so the
            # TensorEngine has work while ScalarE drains exps ----
            osb_tiles = {}

            def emit_qk(c, h):
                """S.T strips + exp + mask for one (tq-chunk, head)."""
                s, po = h // 2, D * (h % 2)
                nj = 4 * c + 4
                pts = []
                for j in range(nj):
                    sps = mm_ps.tile([P, 512], F32, tag="s")
                    pt = ptpool.tile([P, 512], BF16, tag="pt")
                    nc.tensor.matmul(
                        sps[:],
                        KT[po : po + D, s, P * j : P * j + P],
                        QT[po : po + D, s, 512 * c : 512 * c + 512],
                        start=True,
                        stop=True,
                    )
                    nc.scalar.activation(
                        pt[:],
                        sps[:],
                        mybir.ActivationFunctionType.Exp,
                        scale=0.125,
                    )
                    if j >= 4 * c:
                        r = j - 4 * c
                        nc.vector.tensor_mul(pt[:], pt[:], masks4[:, r, :])
                    pts.append(pt)
                return pts

            def emit_pv(c, h, pts):
                """PV accumulate + epilogue for one (tq-chunk, head)."""
                nj = 4 * c + 4
                pv = pv_ps.tile([D + 1, 512], F32, tag="pv")
                for j in range(nj):
                    nc.tensor.matmul(
                        pv[:],
                        VE[:, j, h, :],
                        pts[j][:],
                        start=(j == 0),
                        stop=(j == nj - 1),
                    )
                ot = otpool.tile([D + 1, 512], F32, tag="ot")
                nc.vector.tensor_copy(ot[:], pv[:])
                oc = osb_tiles[c]
                for s4 in range(4):
                    tp = ot_ps.tile([P, D + 1], F32, tag="tp")
                    nc.tensor.transpose(
                        tp[:],
                        ot[:, P * s4 : P * s4 + P],
                        ident[0 : D + 1, 0 : D + 1],
                    )
                    r_ = rpool.tile([P, 1], F32, tag="r")
                    nc.vector.reciprocal(r_[:], tp[:, D : D + 1])
                    nc.vector.tensor_scalar_mul(
                        oc[:, s4, D * h : D * h + D], tp[:, 0:D], r_[:]
                    )

            def emit_out_dma(cc):
                for s4 in range(4):
                    nc.sync.dma_start(
                        out.ap()[
                            512 * cc + P * s4 : 512 * cc + P * s4 + P, :
                        ],
                        osb_tiles[cc][:, s4, :],
                    )

            emit_proj_slice(0)
            pending = None
            for c in range(TC):
                osb_tiles[c] = osb.tile(
                    [P, 4, 512], F32, tag="o", name=f"osb{c}"
                )
                for h in range(HLOC):
                    pts = emit_qk(c, h)
                    if pending is not None:
                        emit_pv(*pending)
                        if pending[1] == HLOC - 1:
                            emit_out_dma(pending[0])
                    pending = (c, h, pts)
                    if h == 4 and c + 1 < TC:
                        emit_proj_slice(c + 1)
            emit_pv(*pending)
            emit_out_dma(pending[0])

    nc.compile()
    return nc


_NC_CACHE = {}


def _get_nc(T):
    if T not in _NC_CACHE:
        _NC_CACHE[T] = build(T)
    return _NC_CACHE[T]


def kernel(inputs_q, inputs_kv, Wq, Wk, Wv):
    inputs_q = np.asarray(inputs_q, dtype=np.float32)
    inputs_kv = np.asarray(inputs_kv, dtype=np.float32)
    Wq = np.asarray(Wq, dtype=np.float32)
    Wk = np.asarray(Wk, dtype=np.float32)
    Wv = np.asarray(Wv, dtype=np.float32)
    T = inputs_q.shape[1]

    bf = ml_dtypes.bfloat16
    in_maps = []
    for c in range(N_CORES):
        b, g = c // 2, c % 2
        sl = slice(g * DLOC, (g + 1) * DLOC)
        in_maps.append(
            {
                "xqT": np.ascontiguousarray(inputs_q[b].T).astype(bf),
                "xkvT": np.ascontiguousarray(inputs_kv[b].T).astype(bf),
                "wqT": np.ascontiguousarray(Wq[sl].T).astype(bf),
                "wkT": np.ascontiguousarray(Wk[sl].T).astype(bf),
                "wvT": np.ascontiguousarray(Wv[sl].T).astype(bf),
            }
        )

    nc = _get_nc(T)
    trace = bool(int(os.environ.get("KERNEL_TRACE", "0")))
    res = run_bass_kernel_spmd(
        nc, in_maps, core_ids=list(range(N_CORES)), trace=trace
    )
    if trace:
        kernel.last_result = res

    full = np.empty((B, T, E), np.float32)
    for c in range(N_CORES):
        b, g = c // 2, c % 2
        full[b, :, g * DLOC : (g + 1) * DLOC] = res.results[c]["out"]
    return full
